# revision 15
# baseline (speedup 1.0000x reference)
"""Trainium2 Bass kernel for nn_Decoder (pointer-network decoder step).

Reference computation (per batch b, sequence position s):
  embedded = decoder_input @ emb_W.T + emb_b                  [B, H]
  h_new    = GRUCell(embedded, last_hidden)                   [B, H]
  pre1[b,s,:]  = attn_W @ concat(static, dynamic, h_new)      [B, S, 3H]
  score1[b,s]  = attn_v . tanh(pre1[b,s,:])
  attn         = softmax_s(score1)
  context[b,:] = sum_s attn[b,s] * static[b,s,:]
  pre2[b,s,:]  = dec_W @ concat(static, context)              [B, S, 2H]
  score2[b,s]  = dec_v . tanh(pre2[b,s,:])
  outputs      = softmax_s(score2)                            [B, S]
  returns (outputs, h_new[None])

Sharding: pure data parallel over batch: B=128 -> 16 per core x 8 cores.
Weights replicated. No collectives.

Per-core layout strategy:
  * Main matmuls run in bf16 with the weight tile stationary and the
    transposed activations streaming, producing pre-activations in a
    [j (3H on partitions), s] orientation. In that orientation the
    per-batch hid/context additive terms are per-partition scalars, so
    they ride along as ACT biases for free.
  * static/dynamic are DMA-loaded naturally [s,H] fp32, converted to
    bf16, and transposed H-major via the DMA xbar transpose.
  * v-dot reductions (contract over j = partitions) run on the PE with
    the v-column stationary, yielding score rows [1, s].
  * S is padded 1000 -> 1024; pad columns are zeroed at the source and
    masked to -30 before exp.
"""

import numpy as np

import concourse.bass as bass
import concourse.mybir as mybir
import concourse.tile as tile
from concourse import bacc
from concourse import bass_utils
from concourse.masks import make_identity

F32 = mybir.dt.float32
BF16 = mybir.dt.bfloat16
AF = mybir.ActivationFunctionType

B, S, H, O = 128, 1000, 256, 2
NCORES = 8
BL = B // NCORES          # 16 batches per core
SP = 1024                 # padded sequence length
NCH = SP // 128           # 8 s-chunks of 128
SVALID_LAST = S - 7 * 128  # 104 valid rows in the last chunk
H3 = 3 * H                # 768
H2 = 2 * H                # 512
NEG = -30.0               # pre-exp mask value for pad columns

INPUT_SHAPES = {
    "decoder_input": (BL, O),
    "last_hidden": (BL, H),
    "static": (BL, S, H),
    "dynamic": (BL, S, H),
    "emb_W": (H, O),
    "emb_b": (H,),
    "gru_Wih": (H3, H),
    "gru_Whh": (H3, H),
    "gru_bih": (H3,),
    "gru_bhh": (H3,),
    "attn_W": (H3, H3),
    "attn_v": (H3,),
    "dec_W": (H2, H2),
    "dec_v": (H2,),
}


def build_kernel(nc: bass.Bass, tc: tile.TileContext, io: dict, ctx):
    static = io["static"]
    dynamic = io["dynamic"]
    outputs = io["outputs"]
    hidden = io["hidden"]

    const = ctx.enter_context(tc.tile_pool(name="const", bufs=1))
    pre = ctx.enter_context(tc.tile_pool(name="psum_pre", bufs=2, space="PSUM"))
    small = ctx.enter_context(tc.tile_pool(name="psum_small", bufs=4, space="PSUM"))

    ident_f = const.tile([128, 128], F32, tag="ident_f")
    make_identity(nc, ident_f)
    ident_b = const.tile([128, 128], BF16, tag="ident_b")
    make_identity(nc, ident_b)
    neg30 = const.tile([1, 1], F32, tag="neg30")
    nc.vector.memset(neg30, NEG)

    def pe_t(out_ps, in_sb, ident):
        # PE transpose: out = in_.T ; identity slice [k, k] with k = in partitions
        k = in_sb.shape[0]
        nc.tensor.transpose(out_ps, in_sb, ident[:k, :k])

    # ---------------- weight preparation ----------------
    # WaT[kt][k=h 128, j 768] bf16 for kt<4 (static/dynamic k-tiles of attn_W.T)
    # W3T[t][k 128, j 768] f32 (hidden k-tiles of attn_W.T, used for hid_term)
    WaT = [const.tile([128, H3], BF16, tag=f"WaT{k}", name=f"WaT{k}") for k in range(4)]
    W3T = [const.tile([128, H3], F32, tag=f"W3T{t}", name=f"W3T{t}") for t in range(2)]
    D1T = [const.tile([128, H2], BF16, tag=f"D1T{k}", name=f"D1T{k}") for k in range(2)]
    D2T = [const.tile([128, H2], BF16, tag=f"D2T{t}", name=f"D2T{t}") for t in range(2)]
    WihT = [const.tile([128, H3], F32, tag=f"WihT{t}", name=f"WihT{t}") for t in range(2)]
    WhhT = [const.tile([128, H3], F32, tag=f"WhhT{t}", name=f"WhhT{t}") for t in range(2)]

    with tc.tile_pool(name="setup", bufs=2) as setup:
        # attn_W [768, 768] -> attn_W.T tiles
        for r in range(6):
            aw = setup.tile([128, H3], F32, tag="aw")
            nc.sync.dma_start(out=aw, in_=io["attn_W"][r * 128:(r + 1) * 128, :])
            for kt in range(6):
                ps = small.tile([128, 128], F32, tag="small")
                pe_t(ps, aw[:, kt * 128:(kt + 1) * 128], ident_f)
                if kt < 4:
                    nc.vector.tensor_copy(out=WaT[kt][:, r * 128:(r + 1) * 128], in_=ps)
                else:
                    nc.vector.tensor_copy(out=W3T[kt - 4][:, r * 128:(r + 1) * 128], in_=ps)
        # dec_W [512, 512] -> dec_W.T tiles
        for r in range(4):
            dw = setup.tile([128, H2], F32, tag="dw")
            nc.sync.dma_start(out=dw, in_=io["dec_W"][r * 128:(r + 1) * 128, :])
            for kt in range(4):
                ps = small.tile([128, 128], F32, tag="small")
                pe_t(ps, dw[:, kt * 128:(kt + 1) * 128], ident_f)
                if kt < 2:
                    nc.vector.tensor_copy(out=D1T[kt][:, r * 128:(r + 1) * 128], in_=ps)
                else:
                    nc.vector.tensor_copy(out=D2T[kt - 2][:, r * 128:(r + 1) * 128], in_=ps)
        # gru_Wih / gru_Whh [768, 256] -> transposed [256, 768]
        for (name, dstT) in (("gru_Wih", WihT), ("gru_Whh", WhhT)):
            for r in range(6):
                w = setup.tile([128, H], F32, tag="wg")
                nc.sync.dma_start(out=w, in_=io[name][r * 128:(r + 1) * 128, :])
                for kt in range(2):
                    ps = small.tile([128, 128], F32, tag="small")
                    pe_t(ps, w[:, kt * 128:(kt + 1) * 128], ident_f)
                    nc.vector.tensor_copy(out=dstT[kt][:, r * 128:(r + 1) * 128], in_=ps)

        # emb_W [256, 2] -> emb_W.T padded to [128, 256] (rows 0..1)
        embWT = const.tile([128, H], F32, tag="embWT")
        nc.vector.memset(embWT, 0.0)
        for t in range(2):
            ew = setup.tile([128, O], F32, tag="ew")
            nc.sync.dma_start(out=ew, in_=io["emb_W"][t * 128:(t + 1) * 128, :])
            ps = small.tile([O, 128], F32, tag="small")
            pe_t(ps, ew, ident_f)
            nc.vector.tensor_copy(out=embWT[0:O, t * 128:(t + 1) * 128], in_=ps)

        # v / dec_v as column tiles (bf16), even columns for 4B alignment
        vT = const.tile([128, 12], BF16, tag="vT")
        vf = setup.tile([128, 6], F32, tag="vf")
        nc.sync.dma_start(out=vf, in_=io["attn_v"].rearrange("(t p) -> p t", p=128))
        nc.vector.tensor_copy(out=vT.rearrange("p (t two) -> p t two", two=2)[:, :, 0],
                              in_=vf)
        dvT = const.tile([128, 8], BF16, tag="dvT")
        dvf = setup.tile([128, 4], F32, tag="dvf")
        nc.sync.dma_start(out=dvf, in_=io["dec_v"].rearrange("(t p) -> p t", p=128))
        nc.vector.tensor_copy(out=dvT.rearrange("p (t two) -> p t two", two=2)[:, :, 0],
                              in_=dvf)

    # ---------------- GRU (fp32, one-time) ----------------
    # decoder_input.T padded [128, 16]
    dT = const.tile([128, BL], F32, tag="dT")
    nc.vector.memset(dT, 0.0)
    d_nat = const.tile([BL, O], F32, tag="d_nat")
    nc.sync.dma_start(out=d_nat, in_=io["decoder_input"])
    ps = small.tile([O, BL], F32, tag="small")
    pe_t(ps, d_nat, ident_f)
    nc.vector.tensor_copy(out=dT[0:O, :], in_=ps)

    # embedded = decoder_input @ emb_W.T + emb_b
    ep = pre.tile([BL, H], F32, tag="pre")
    nc.tensor.matmul(ep, lhsT=dT, rhs=embWT, start=True, stop=True)
    ebb = const.tile([BL, H], F32, tag="ebb")
    nc.gpsimd.dma_start(out=ebb, in_=io["emb_b"].partition_broadcast(BL))
    emb_sb = const.tile([BL, H], F32, tag="emb_sb")
    nc.vector.tensor_add(out=emb_sb, in0=ep, in1=ebb)

    def transpose_16xH(src_sb, tag):
        # [16, 256] f32 -> two [128, 16] f32 column tiles
        outs = []
        for t in range(2):
            tp = small.tile([128, BL], F32, tag="small")
            pe_t(tp, src_sb[:, t * 128:(t + 1) * 128], ident_f)
            dst = const.tile([128, BL], F32, tag=f"{tag}{t}", name=f"{tag}{t}")
            nc.vector.tensor_copy(out=dst, in_=tp)
            outs.append(dst)
        return outs

    embT = transpose_16xH(emb_sb, "embT")

    h_sb = const.tile([BL, H], F32, tag="h_sb")
    nc.sync.dma_start(out=h_sb, in_=io["last_hidden"])
    lhT = transpose_16xH(h_sb, "lhT")

    bx_bc = const.tile([BL, H3], F32, tag="bx_bc")
    nc.gpsimd.dma_start(out=bx_bc, in_=io["gru_bih"].partition_broadcast(BL))
    bh_bc = const.tile([BL, H3], F32, tag="bh_bc")
    nc.gpsimd.dma_start(out=bh_bc, in_=io["gru_bhh"].partition_broadcast(BL))

    def gates(xT, WT, bias_bc, tag):
        g = const.tile([BL, H3], F32, tag=tag)
        for hf in range(2):
            sl = slice(hf * 384, (hf + 1) * 384)
            gp = pre.tile([BL, 384], F32, tag="pre")
            for t in range(2):
                nc.tensor.matmul(gp, lhsT=xT[t], rhs=WT[t][:, sl],
                                 start=(t == 0), stop=(t == 1))
            nc.vector.tensor_add(out=g[:, sl], in0=gp, in1=bias_bc[:, sl])
        return g

    gx = gates(embT, WihT, bx_bc, "gx")
    gh = gates(lhT, WhhT, bh_bc, "gh")

    r_sb = const.tile([BL, H], F32, tag="r_sb")
    nc.vector.tensor_add(out=r_sb, in0=gx[:, 0:H], in1=gh[:, 0:H])
    nc.scalar.activation(out=r_sb, in_=r_sb, func=AF.Sigmoid)
    z_sb = const.tile([BL, H], F32, tag="z_sb")
    nc.vector.tensor_add(out=z_sb, in0=gx[:, H:2 * H], in1=gh[:, H:2 * H])
    nc.scalar.activation(out=z_sb, in_=z_sb, func=AF.Sigmoid)
    n_sb = const.tile([BL, H], F32, tag="n_sb")
    nc.vector.tensor_mul(out=n_sb, in0=r_sb, in1=gh[:, 2 * H:3 * H])
    nc.vector.tensor_add(out=n_sb, in0=n_sb, in1=gx[:, 2 * H:3 * H])
    nc.scalar.activation(out=n_sb, in_=n_sb, func=AF.Tanh)
    # h_new = n + z * (h - n)
    hn_sb = const.tile([BL, H], F32, tag="hn_sb")
    nc.vector.tensor_sub(out=hn_sb, in0=h_sb, in1=n_sb)
    nc.vector.tensor_mul(out=hn_sb, in0=z_sb, in1=hn_sb)
    nc.vector.tensor_add(out=hn_sb, in0=n_sb, in1=hn_sb)
    nc.sync.dma_start(out=hidden, in_=hn_sb)

    # hid_term = h_new @ W3.T  [16, 768]
    hnT = transpose_16xH(hn_sb, "hnT")
    hid_sb = const.tile([BL, H3], F32, tag="hid_sb")
    for hf in range(2):
        sl = slice(hf * 384, (hf + 1) * 384)
        hp = pre.tile([BL, 384], F32, tag="pre")
        for t in range(2):
            nc.tensor.matmul(hp, lhsT=hnT[t], rhs=W3T[t][:, sl],
                             start=(t == 0), stop=(t == 1))
        nc.vector.tensor_copy(out=hid_sb[:, sl], in_=hp)
    # hidT columns: [128, 16] per j-tile, used as per-partition ACT bias
    hidT = []
    for j in range(6):
        tp = small.tile([128, BL], F32, tag="small")
        pe_t(tp, hid_sb[:, j * 128:(j + 1) * 128], ident_f)
        dst = const.tile([128, BL], F32, tag=f"hidT{j}", name=f"hidT{j}")
        nc.vector.tensor_copy(out=dst, in_=tp)
        hidT.append(dst)

    # ---------------- main per-batch loop ----------------
    stpool = ctx.enter_context(tc.tile_pool(name="st", bufs=2))
    dypool = ctx.enter_context(tc.tile_pool(name="dy", bufs=2))
    ldpool = ctx.enter_context(tc.tile_pool(name="ld", bufs=4))
    tanhpool = ctx.enter_context(tc.tile_pool(name="tanh", bufs=3))
    rowpool = ctx.enter_context(tc.tile_pool(name="row", bufs=3))

    for b in range(BL):
        # --- load + convert + transpose static/dynamic ---
        st_bf = stpool.tile([128, NCH, H], BF16, tag="st_bf")
        stT = stpool.tile([128, 2, SP], BF16, tag="stT")
        dy_bf = dypool.tile([128, NCH, H], BF16, tag="dy_bf")
        dyT = dypool.tile([128, 2, SP], BF16, tag="dyT")
        for (src, bf) in ((static, st_bf), (dynamic, dy_bf)):
            nat = ldpool.tile([128, NCH, H], F32, tag="nat")
            nc.sync.dma_start(
                out=nat[:, 0:NCH - 1, :],
                in_=src[b, 0:896, :].rearrange("(c p) h -> p c h", p=128))
            nc.sync.dma_start(out=nat[0:SVALID_LAST, NCH - 1, :],
                              in_=src[b, 896:S, :])
            nc.vector.tensor_copy(out=bf[:, 0:NCH - 1, :], in_=nat[:, 0:NCH - 1, :])
            nc.vector.memset(bf[96:128, NCH - 1, :], 0.0)
            nc.vector.tensor_copy(out=bf[0:SVALID_LAST, NCH - 1, :],
                                  in_=nat[0:SVALID_LAST, NCH - 1, :])
        for (bf, tT) in ((st_bf, stT), (dy_bf, dyT)):
            for c in range(NCH):
                nc.sync.dma_start_transpose(
                    out=tT[:, :, c * 128:(c + 1) * 128], in_=bf[:, c, :])

        # --- attention scores ---
        score_ps = [small.tile([1, 512], F32, tag="small", name=f"score_ps{b}_{i}") for i in range(2)]
        for j in range(6):
            pre_ps = pre.tile([128, SP], F32, tag="pre")
            for kt in range(4):
                for sh in range(2):
                    ssl = slice(sh * 512, (sh + 1) * 512)
                    rhs = (stT if kt < 2 else dyT)[:, kt % 2, ssl]
                    nc.tensor.matmul(pre_ps[:, ssl],
                                     lhsT=WaT[kt][:, j * 128:(j + 1) * 128],
                                     rhs=rhs, start=(kt == 0), stop=(kt == 3))
            th = tanhpool.tile([128, SP], BF16, tag="tanh")
            nc.scalar.activation(out=th, in_=pre_ps, func=AF.Tanh,
                                 bias=hidT[j][:, b:b + 1])
            for sh in range(2):
                nc.tensor.matmul(score_ps[sh], lhsT=vT[:, 2 * j:2 * j + 1],
                                 rhs=th[:, sh * 512:(sh + 1) * 512],
                                 start=(j == 0), stop=(j == 5))

        # --- softmax pieces (no max-subtraction; scores are small) ---
        nc.scalar.activation(out=score_ps[1][:, S - 512:512],
                             in_=score_ps[1][:, S - 512:512],
                             func=AF.Identity, scale=0.0, bias=neg30[:, 0:1])
        exp_row = rowpool.tile([1, SP], BF16, tag="exp_row")
        denp = rowpool.tile([1, 2], F32, tag="denp")
        for sh in range(2):
            nc.scalar.activation(out=exp_row[:, sh * 512:(sh + 1) * 512],
                                 in_=score_ps[sh], func=AF.Exp,
                                 accum_out=denp[:, sh:sh + 1])
        den = rowpool.tile([1, 2], F32, tag="den")
        nc.vector.tensor_add(out=den[:, 0:1], in0=denp[:, 0:1], in1=denp[:, 1:2])
        nc.vector.reciprocal(out=den[:, 1:2], in_=den[:, 0:1])

        colT_ps = small.tile([128, 2 * NCH], BF16, tag="small")
        for c in range(NCH):
            pe_t(colT_ps[:, 2 * c:2 * c + 1], exp_row[:, c * 128:(c + 1) * 128], ident_b)
        attn_col = rowpool.tile([128, 2 * NCH], BF16, tag="attn_col")
        nc.vector.tensor_copy(
            out=attn_col.rearrange("p (c two) -> p c two", two=2)[:, :, 0],
            in_=colT_ps.rearrange("p (c two) -> p c two", two=2)[:, :, 0])

        # --- context = (sum_s exp[s] * static[s, :]) / den ---
        ctx_ps = small.tile([1, H], F32, tag="small")
        for c in range(NCH):
            nc.tensor.matmul(ctx_ps, lhsT=attn_col[:, 2 * c:2 * c + 1], rhs=st_bf[:, c, :],
                             start=(c == 0), stop=(c == NCH - 1))
        ctx_row = rowpool.tile([1, H], BF16, tag="ctx_row")
        nc.vector.tensor_scalar_mul(ctx_row, in0=ctx_ps, scalar1=den[:, 1:2])
        ctxT_ps = small.tile([128, 4], BF16, tag="small")
        for t in range(2):
            pe_t(ctxT_ps[:, 2 * t:2 * t + 1], ctx_row[:, t * 128:(t + 1) * 128], ident_b)
        ctxT = rowpool.tile([128, 4], BF16, tag="ctxT")
        nc.vector.tensor_copy(
            out=ctxT.rearrange("p (t two) -> p t two", two=2)[:, :, 0],
            in_=ctxT_ps.rearrange("p (t two) -> p t two", two=2)[:, :, 0])

        # ctx_term = context @ D2.T  [1, 512] -> columns [128, 4] f32
        ctt_ps = small.tile([1, H2], F32, tag="small")
        for t in range(2):
            nc.tensor.matmul(ctt_ps, lhsT=ctxT[:, 2 * t:2 * t + 1], rhs=D2T[t],
                             start=(t == 0), stop=(t == 1))
        ctt_row = rowpool.tile([1, H2], BF16, tag="ctt_row")
        nc.scalar.copy(out=ctt_row, in_=ctt_ps)
        cttT_ps = small.tile([128, 8], BF16, tag="small")
        for q in range(4):
            pe_t(cttT_ps[:, 2 * q:2 * q + 1], ctt_row[:, q * 128:(q + 1) * 128], ident_b)
        ctt = rowpool.tile([128, 4], F32, tag="ctt")
        nc.vector.tensor_copy(
            out=ctt,
            in_=cttT_ps.rearrange("p (q two) -> p q two", two=2)[:, :, 0])

        # --- pointer scorer ---
        score2_ps = [small.tile([1, 512], F32, tag="small", name=f"score2_ps{b}_{i}") for i in range(2)]
        for j2 in range(4):
            pre2_ps = pre.tile([128, SP], F32, tag="pre")
            for kt in range(2):
                for sh in range(2):
                    ssl = slice(sh * 512, (sh + 1) * 512)
                    nc.tensor.matmul(pre2_ps[:, ssl],
                                     lhsT=D1T[kt][:, j2 * 128:(j2 + 1) * 128],
                                     rhs=stT[:, kt, ssl],
                                     start=(kt == 0), stop=(kt == 1))
            th2 = tanhpool.tile([128, SP], BF16, tag="tanh")
            nc.scalar.activation(out=th2, in_=pre2_ps, func=AF.Tanh,
                                 bias=ctt[:, j2:j2 + 1])
            for sh in range(2):
                nc.tensor.matmul(score2_ps[sh], lhsT=dvT[:, 2 * j2:2 * j2 + 1],
                                 rhs=th2[:, sh * 512:(sh + 1) * 512],
                                 start=(j2 == 0), stop=(j2 == 3))

        nc.scalar.activation(out=score2_ps[1][:, S - 512:512],
                             in_=score2_ps[1][:, S - 512:512],
                             func=AF.Identity, scale=0.0, bias=neg30[:, 0:1])
        exp2_row = rowpool.tile([1, SP], F32, tag="exp2_row")
        den2p = rowpool.tile([1, 2], F32, tag="den2p")
        for sh in range(2):
            nc.scalar.activation(out=exp2_row[:, sh * 512:(sh + 1) * 512],
                                 in_=score2_ps[sh], func=AF.Exp,
                                 accum_out=den2p[:, sh:sh + 1])
        den2 = rowpool.tile([1, 2], F32, tag="den2")
        nc.vector.tensor_add(out=den2[:, 0:1], in0=den2p[:, 0:1], in1=den2p[:, 1:2])
        nc.vector.reciprocal(out=den2[:, 1:2], in_=den2[:, 0:1])
        out_row = rowpool.tile([1, SP], F32, tag="out_row")
        nc.vector.tensor_scalar_mul(out_row[:, 0:S], in0=exp2_row[:, 0:S],
                                    scalar1=den2[:, 1:2])
        nc.sync.dma_start(out=outputs[b:b + 1, :], in_=out_row[:, 0:S])


_CACHE = {}


def _build():
    if "nc" in _CACHE:
        return _CACHE["nc"]
    nc = bacc.Bacc("TRN2", target_bir_lowering=False, debug=False,
                   enable_asserts=True, num_devices=NCORES)
    io = {}
    for name, shape in INPUT_SHAPES.items():
        io[name] = nc.dram_tensor(name, list(shape), F32, kind="ExternalInput").ap()
    io["outputs"] = nc.dram_tensor("outputs", [BL, S], F32, kind="ExternalOutput").ap()
    io["hidden"] = nc.dram_tensor("hidden", [BL, H], F32, kind="ExternalOutput").ap()
    from contextlib import ExitStack
    with tile.TileContext(nc) as tc:
        with ExitStack() as ctx:
            build_kernel(nc, tc, io, ctx)
    nc.compile()
    _CACHE["nc"] = nc
    return nc


def make_in_maps(inputs):
    arrs = {k: np.asarray(v, dtype=np.float32) for k, v in inputs.items()}
    in_maps = []
    for c in range(NCORES):
        bsl = slice(c * BL, (c + 1) * BL)
        m = {}
        for name in INPUT_SHAPES:
            if name in ("decoder_input", "last_hidden", "static", "dynamic"):
                m[name] = np.ascontiguousarray(arrs[name][bsl])
            else:
                m[name] = arrs[name]
        in_maps.append(m)
    return in_maps


def kernel(**inputs):
    nc = _build()
    in_maps = make_in_maps(inputs)
    res = bass_utils.run_bass_kernel_spmd(nc, in_maps, core_ids=list(range(NCORES)))
    outputs = np.concatenate([res.results[c]["outputs"] for c in range(NCORES)], axis=0)
    hidden = np.concatenate([res.results[c]["hidden"] for c in range(NCORES)], axis=0)
    return outputs.astype(np.float32), hidden[None].astype(np.float32)
